# revision 44
# baseline (speedup 1.0000x reference)
"""MoE experts kernel (grouped GEMM + SwiGLU) on 8 Trainium2 NeuronCores.

Problem: N=4096 tokens sorted by expert, E=8 experts, H=1024, I=2048, bf16.
  up    = gmm(hiddens, w13)            # [N, 2I]
  gated = silu(up[:, :I]) * up[:, I:]  # [N, I]
  down  = gmm(gated, w2)               # [N, H]

Sharding: expert parallelism. Core e owns expert e's weights and its
contiguous block of tokens (batch_sizes[e] each; 512 in the target
regime). No collectives; tokens are scattered/gathered on the host.

Per-core dataflow (lhsT = stationary operand of nc.tensor.matmul):
  xT   [H, T] <- PE transpose of x (identity shipped from host inside x)
  upT  = matmul(lhsT=w13[:, chunk], rhs=xT)  -> PSUM [128, T]   (k = H)
  gatedT[c] = silu(upT_gate) * upT_up        -> SBUF bf16 chunks
  down = matmul(lhsT=gatedT[c], rhs=w2[c])   -> PSUM [128, 512] (k = I)
Both weights are consumed in native K-major DRAM layout; only x needs a
transpose, done on the PE against a host-supplied identity block.

Environment constraint that shaped everything here: this walrus build
rejects ANY instruction carrying more than one embedded sync wait. Hence
- all loads ride the single SWDGE lane (strict FIFO = priority order,
  zero waits on load DMAs),
- dummy PE "observer" transposes absorb DMA-progress waits so real
  matmuls only ever carry one wait,
- a custom TileContext splits the tail drain into one-wait chains.
"""

import sys

if "/opt/trn_rl_repo" not in sys.path:
    sys.path.insert(0, "/opt/trn_rl_repo")

import numpy as np
import ml_dtypes

E = 8
H = 1024
I = 2048
N = 4096
T = N // E          # tokens per expert / core
P = 128
XA = T + P          # x input is augmented with 128 identity rows
KH = H // P         # 8  k-subtiles for mm1
NI = I // P         # 16 k-subtiles for mm2 / gated chunks
FD = 512            # matmul moving free dim (1 PSUM bank of f32)
# w13 column-slab widths per half: small first so the first pairs start
# early while the SWDGE FIFO streams the rest.
SLABS = (128, 384, 512, 512, 512)
BF16 = ml_dtypes.bfloat16

_NC_CACHE = {}


def _slab_of(c):
    """Map gated-chunk index c (0..15) -> (slab_idx, col offset in slab)."""
    base = 0
    for si, w in enumerate(SLABS):
        n = w // P
        if c < n:
            return si, c * P
        c -= n
        base += w
    raise IndexError(c)


def _build_nc(act="silu"):
    import concourse.bass as bass
    import concourse.tile as tile
    from concourse import mybir
    from concourse.vector_clock import ScopedClock, VectorClock

    class SplitDrainTileContext(tile.TileContext):
        """Tail drain emitted as a chain of single-wait drains (the
        compiler rejects instructions with >1 embedded sync wait)."""

        def _drain_and_barrier(self, tick_clock, wait_clock):
            # Emit the quiescence chain on gpsimd — the same engine that
            # clear_and_free_semaphores uses for its sem_clear ops — so
            # plain program order replaces both EVSEM all-engine
            # barriers (~3us each). Each drain carries exactly one wait.
            nc = self.nc
            gclock = tick_clock.global_clock
            n = len(gclock)
            for p in range(n):
                if gclock[p] <= 0:
                    continue
                masked = VectorClock([gclock[q] if q == p else 0
                                      for q in range(n)])
                d = nc.gpsimd.drain()
                wait_clock.add_sem_waits(d.ins, ScopedClock({None: masked}))
            # One EVSEM barrier before the clears (engines arrive idle
            # while gpsimd drains, so it mostly overlaps the chain). The
            # post-clear barrier is dropped: the runtime already waits
            # for every engine to halt before the NEFF completes.
            nc.all_engine_barrier()
            assert self.sems is not None
            popped = nc._tile_sem_poison_stack.pop()
            assert popped is self._sem_poison
            nc.clear_and_free_semaphores(list(self.sems.allocated().values()))

    # 4 SWDGE lanes: consecutive bulk loads pipeline across lanes instead
    # of paying a full completion round-trip between every transfer.
    nc = bass.Bass(num_swdge_queues=4)
    bf = mybir.dt.bfloat16
    f32 = mybir.dt.float32

    # All inputs arrive pre-arranged on the host into partition-major
    # layouts so every load DMA is contiguous per partition (128 fat
    # descriptors instead of 1024+ thin ones; SWDGE descriptor
    # generation becomes negligible and transfers run at full HBM BW).
    #   x:   [P, (XA//P)*H]   block 0 = identity rows, blocks 1.. = tokens
    #   w13: [P, KH*2I]       slabs concatenated in consumption order
    #   w2:  [P, NI*H]
    x = nc.declare_dram_parameter("x", [P, (XA // P) * H], bf, isOutput=False)
    w13 = nc.declare_dram_parameter("w13", [P, KH * 2 * I], bf, isOutput=False)
    w2 = nc.declare_dram_parameter("w2", [P, NI * H], bf, isOutput=False)
    out = nc.declare_dram_parameter("out", [T, H], bf, isOutput=True)

    fn = (mybir.ActivationFunctionType.Silu if act == "silu"
          else mybir.ActivationFunctionType.Sigmoid)

    with SplitDrainTileContext(nc) as tc:
        with (
            tc.tile_pool(name="persist", bufs=1) as persist,
            tc.tile_pool(name="sgp", bufs=16) as sgp,
            tc.tile_pool(name="gtp", bufs=16) as gtp,
            tc.tile_pool(name="tch", bufs=16) as tch,
            tc.tile_pool(name="otp", bufs=1) as otp,
            tc.tile_pool(name="pst", bufs=2, space="PSUM") as pst,
            tc.tile_pool(name="ps1", bufs=2, space="PSUM") as ps1,
            tc.tile_pool(name="ps2", bufs=2, space="PSUM") as ps2,
        ):
            # ---- Load plan ----
            # Early-critical loads (x, slabs 0-1) go on HWDGE queues:
            # with fair HBM sharing, completion time is proportional to
            # size, so these small loads finish first. The bulk (slabs
            # 2-4, w2) rides the single SWDGE lane, whose strict FIFO
            # defers it behind nothing and keeps it ordered.
            xrows = persist.tile([P, XA // P, H], bf)
            xv = x.rearrange("p (a h) -> p a h", h=H)
            nc.sync.dma_start(xrows[:, 0:2, :], xv[:, 0:2, :])
            nc.sync.dma_start(xrows[:, 2:5, :], xv[:, 2:5, :])
            ident = xrows[:, 0, 0:P]

            w13g_slabs = [None] * len(SLABS)
            w13u_slabs = [None] * len(SLABS)
            off = 0
            for si, wdt in enumerate(SLABS):
                g = persist.tile([P, KH, wdt], bf, tag=f"w13g{si}")
                u = persist.tile([P, KH, wdt], bf, tag=f"w13u{si}")
                eng = nc.sync if si < 1 else nc.gpsimd
                seg = KH * wdt
                eng.dma_start(
                    g[:], w13[:, off:off + seg].rearrange(
                        "p (o m) -> p o m", m=wdt)
                )
                eng.dma_start(
                    u[:], w13[:, off + seg:off + 2 * seg].rearrange(
                        "p (o m) -> p o m", m=wdt)
                )
                w13g_slabs[si] = g
                w13u_slabs[si] = u
                off += 2 * seg

            w2s = persist.tile([P, NI, H], bf)
            nc.gpsimd.dma_start(
                w2s[:], w2.rearrange("p (o h) -> p o h", h=H)
            )

            # ---- x transpose on PE: 32x 128x128 against host identity ----
            # Token block a lives at xrows[:, 1 + a] (block 0 is identity).
            # Four transposes land in one PSUM bank so DVE does one wide
            # copy per H-chunk instead of four narrow ones.
            xT = persist.tile([P, KH, T], bf)
            for o in range(KH):
                for a in range(T // P):
                    pt = pst.tile([P, T], bf, tag="ptb")
                    nc.tensor.transpose(
                        pt[:, 0:P], xrows[:, 1 + a, o * P:(o + 1) * P], ident
                    )
                    nc.vector.tensor_copy(
                        xT[:, o, a * P:(a + 1) * P], pt[:, 0:P]
                    )

            # ---- mm1 + SwiGLU over 16 gate/up column-chunk pairs ----
            gts = []
            seen_slab = -1
            for c in range(NI):
                si, co = _slab_of(c)
                if si != seen_slab:
                    # Observer transposes: absorb the DMA-progress wait
                    # for this slab (g and u) on the PE so the real
                    # matmuls below carry only their single WAR wait.
                    for src in (w13g_slabs[si], w13u_slabs[si]):
                        pt = pst.tile([P, T], bf, tag="ptb")
                        nc.tensor.transpose(pt[:, 0:P], src[:, 0, 0:P], ident)
                    seen_slab = si
                pg = ps1.tile([P, T], f32, tag="pg")
                pu = ps1.tile([P, T], f32, tag="pu")
                for k in range(KH):
                    nc.tensor.matmul(
                        pg[:], w13g_slabs[si][:, k, co:co + P], xT[:, k, :],
                        start=(k == 0), stop=(k == KH - 1),
                    )
                for k in range(KH):
                    nc.tensor.matmul(
                        pu[:], w13u_slabs[si][:, k, co:co + P], xT[:, k, :],
                        start=(k == 0), stop=(k == KH - 1),
                    )
                sg = sgp.tile([P, T], bf, tag="sg")
                nc.scalar.activation(sg[:], pg[:], fn)
                # A DVE instruction may carry one sync wait: this tiny copy
                # takes the ACT wait so the gating mul below only needs PE.
                touch = tch.tile([P, 1], bf, tag="touch")
                nc.vector.tensor_copy(touch[:], sg[:, 0:1])
                gt = gtp.tile([P, T], bf, tag="gt")
                nc.vector.scalar_tensor_tensor(
                    gt[:], pu[:], 1.0, sg[:],
                    mybir.AluOpType.mult, mybir.AluOpType.mult,
                )
                gts.append(gt)

            # Observers for w2s before mm2 reads it (one per SWDGE lane
            # sem that mm2's operands depend on would be ideal; w2s is a
            # single DMA so one suffices).
            pt = pst.tile([P, T], bf, tag="ptb")
            nc.tensor.transpose(pt[:, 0:P], w2s[:, 0, 0:P], ident)

            # ---- mm2: down[mc*P:, :] = gatedT.T @ w2 ----
            obuf = otp.tile([P, T // P, H], bf, tag="obuf")
            for mc in range(T // P):  # 4
                for nh in range(H // FD):  # 2
                    pd = ps2.tile([P, FD], f32, tag="pd")
                    for kc in range(NI):
                        nc.tensor.matmul(
                            pd[:],
                            gts[kc][:, mc * P:(mc + 1) * P],
                            w2s[:, kc, nh * FD:(nh + 1) * FD],
                            start=(kc == 0), stop=(kc == NI - 1),
                        )
                    nc.vector.tensor_copy(
                        obuf[:, mc, nh * FD:(nh + 1) * FD], pd[:]
                    )
                if mc == 2:
                    nc.sync.dma_start(
                        out.rearrange("(a p) h -> p a h", p=P)[:, 0:3, :],
                        obuf[:, 0:3, :],
                    )
            nc.sync.dma_start(
                out.rearrange("(a p) h -> p a h", p=P)[:, 3:4, :],
                obuf[:, 3:4, :],
            )

    return nc


def _get_nc():
    if "nc" not in _NC_CACHE:
        _NC_CACHE["nc"] = _build_nc()
    return _NC_CACHE["nc"]


def _ident_rows():
    ir = np.zeros((P, H), dtype=BF16)
    ir[:P, :P] = np.eye(P, dtype=np.float32).astype(BF16)
    return ir


def _prep_x(tokens):
    """tokens [T, H] -> [P, (XA//P)*H]: identity block then token blocks,
    partition-major so the load is one 10KB-per-partition descriptor."""
    blocks = np.concatenate(
        [_ident_rows()[None], tokens.reshape(T // P, P, H)], axis=0
    )  # [5, P, H]
    return np.ascontiguousarray(
        blocks.transpose(1, 0, 2).reshape(P, (XA // P) * H)
    )


def _prep_w13(w13_e):
    """w13_e [H, 2I] -> [P, KH*2I]: per-partition concatenation of
    (gate-slab, up-slab) pairs in kernel consumption order."""
    w4 = w13_e.reshape(KH, P, 2 * I)
    parts = []
    off = 0
    for wdt in SLABS:
        parts.append(w4[:, :, off:off + wdt])          # gate slab
        parts.append(w4[:, :, I + off:I + off + wdt])  # up slab
        off += wdt
    flat = np.concatenate(
        [p.transpose(1, 0, 2).reshape(P, -1) for p in parts], axis=1
    )
    return np.ascontiguousarray(flat)


def _prep_w2(w2_e):
    """w2_e [I, H] -> [P, NI*H] partition-major."""
    return np.ascontiguousarray(
        w2_e.reshape(NI, P, H).transpose(1, 0, 2).reshape(P, NI * H)
    )


def kernel(bs, hiddens, w13_weight, w2_weight, batch_sizes, **_ignored):
    from concourse.bass_utils import run_bass_kernel_spmd

    hiddens = np.asarray(hiddens)
    w13_weight = np.asarray(w13_weight)
    w2_weight = np.asarray(w2_weight)
    batch_sizes = np.asarray(batch_sizes).astype(np.int64)

    in_dtype = hiddens.dtype
    x = np.ascontiguousarray(hiddens.astype(BF16))
    w13 = np.ascontiguousarray(w13_weight.astype(BF16))
    w2 = np.ascontiguousarray(w2_weight.astype(BF16))

    assert batch_sizes.shape == (E,) and int(batch_sizes.sum()) == N, (
        "kernel compiled for 8 experts x 4096 tokens"
    )

    offsets = np.concatenate([[0], np.cumsum(batch_sizes)])
    uniform = bool((batch_sizes == T).all())

    in_maps = []
    for e in range(E):
        if uniform:
            tok = x[e * T:(e + 1) * T]
        else:
            blk = x[offsets[e]:offsets[e + 1]]
            assert blk.shape[0] <= T, "per-expert batch exceeds compiled T"
            tok = np.zeros((T, H), dtype=BF16)
            tok[: blk.shape[0]] = blk
        in_maps.append({"x": _prep_x(tok),
                        "w13": _prep_w13(w13[e]),
                        "w2": _prep_w2(w2[e])})

    nc = _get_nc()
    results = run_bass_kernel_spmd(nc, in_maps, list(range(E))).results

    out_full = np.empty((N, H), dtype=BF16)
    for e in range(E):
        oe = np.asarray(results[e]["out"])
        if uniform:
            out_full[e * T:(e + 1) * T] = oe
        else:
            nb = int(batch_sizes[e])
            out_full[offsets[e]:offsets[e + 1]] = oe[:nb]

    return out_full.astype(in_dtype)
